# revision 1
# baseline (speedup 1.0000x reference)
"""Trainium2 Bass kernel for nn_ContrastiveEmbeddingLoss (N=8192, D=128).

Strategy (sharding_hint): anchors (rows of the NxN similarity matrix) are
sharded across the 8 NeuronCores; each core computes its [1024, 8192] sim
block against the replicated embeddings, reduces its partial loss sum, and
the host combines the 8 partial scalars.

Algorithmic layout: labels take only values {-1, 0, 1}, so rows/columns are
permuted on the host to sort by label.  The pos/neg masked sums over exp(sim)
then become *contiguous range* sums over 3 column ranges.  Per anchor i:

    S[c]  = sum_{j in class c} exp(sim_ij - m_i)   (m_i = true row max)
    P_i   = sum_{c in pos(label_i)} S[c] - exp(sim_ii - m_i)
    G_i   = sum_{c in neg(label_i)} S[c]
    loss_i = log(P_i + G_i + EPS) - log(P_i + EPS)  ( = -log((P+eps)/(T+eps)) )

Single matmul pass (flash-softmax style): each 128-anchor block is computed
once as 9 class-aligned chunks of <=1024 columns (4 rotating 2-bank PSUM
tiles keep the MM -> max -> exp 3-stage chunk pipeline full).  Per chunk the VectorE
reduces the chunk row-max from PSUM (negated), and ScalarE computes
exp(raw - chunk_max) with a fused accumulate-sum.  At the end of the block
the true row max M = max_k m_k and each chunk sum is rescaled by
exp(m_k - M) -- mathematically exact, and exp arguments never exceed 0.
The diagonal self-term is extracted via a mini-matmul + identity mask and
rescaled with the same per-chunk factor (bitwise-identical to its in-sum
contribution, so P is an exact cancellation).

The 1/T=2.0 temperature factor is folded into the lhsT operand on the host
(anch2 = 2 * bf16(E^T) -- exact scaling), so matmuls produce sim directly.
"""

import numpy as np

N, D = 8192, 128
NCORES = 8
RPC = N // NCORES        # anchor rows per core
BPB = 128                # anchors per block (= partition dim)
NBLK = RPC // BPB        # blocks per core
TEMPERATURE = 0.5
EPS = 1e-08
CHUNK = 1024             # pass chunk width (2 PSUM banks)
MM_MAX = 512             # max matmul free dim (one PSUM bank)

LAST_RESULT = None       # BassKernelResults of the most recent run (for test.py)


def _split_drain_tile_context(tile_mod, mybir, ScopedClock):
    """TileContext subclass that never emits more than one sync wait per
    instruction -- the walrus build here rejects any instruction carrying
    more than one ("Too many sync wait commands").  Excess waits are hoisted
    onto same-engine NoOp instructions inserted immediately before (engine
    program order makes sequential single waits equivalent to one multi-wait:
    a logical AND), and the tail drain is split into sequential drains."""

    class SplitWaitTileContext(tile_mod.TileContext):
        def _lower_ordered_insts(self, ordered):
            unassigned = mybir.EngineType.Unassigned
            for insts in ordered.values():
                new_list = []
                changed = False
                for inst in insts:
                    si = inst.sync_info
                    waits = list(si.on_wait) if si is not None and si.on_wait else []
                    eng = getattr(inst, "engine", None)
                    if len(waits) > 1 and eng is not None and eng != unassigned:
                        keep = [w for w in waits if w.sync_type != "semaphore"]
                        move = [w for w in waits if w.sync_type == "semaphore"]
                        if not keep and move:
                            keep = [move.pop()]
                        for w in move:
                            nop = mybir.InstNoOp(
                                name=f"I-{self.nc.next_id()}", ins=[], outs=[]
                            )
                            nop.engine = eng
                            nop.sync_info = mybir.SyncInfo(
                                on_wait=[w], on_update=[]
                            )
                            new_list.append(nop)
                        inst.sync_info = mybir.SyncInfo(
                            on_wait=keep,
                            on_update=list(si.on_update) if si.on_update else [],
                        )
                        changed = True
                    new_list.append(inst)
                if changed:
                    insts[:] = new_list
            return super()._lower_ordered_insts(ordered)

        def _drain_and_barrier(self, tick_clock, wait_clock):
            nc = self.nc
            drain_inst = nc.sync.drain()
            wait_clock.add_sem_waits(
                drain_inst.ins, ScopedClock({None: tick_clock.global_clock})
            )
            si = drain_inst.ins.sync_info
            waits = list(si.on_wait) if si is not None and si.on_wait else []
            if len(waits) > 1:
                drain_inst.ins.sync_info = mybir.SyncInfo(
                    on_wait=waits[:1],
                    on_update=list(si.on_update) if si.on_update else [],
                )
                for i in range(1, len(waits)):
                    extra = nc.sync.drain()
                    extra.ins.sync_info = mybir.SyncInfo(
                        on_wait=waits[i : i + 1], on_update=[]
                    )
            # Single-shot NEFF: skip the semaphore-clearing pass + second
            # barrier (cleanup for NEFF re-execution, which never happens
            # here -- each kernel() call compiles and runs a fresh NEFF).
            # The drain above already waits for the output DMA; the barrier
            # aligns all engines before the program ends.
            nc.all_engine_barrier()
            assert self.sems is not None
            popped = nc._tile_sem_poison_stack.pop()
            assert popped is self._sem_poison
            # Sems are intentionally NOT cleared or returned to the pool:
            # this is the outermost (only) TileContext of a one-shot program,
            # so nothing after it allocates semaphores.

    return SplitWaitTileContext


def _build_program(chunks):
    """Build the SPMD Bass program.  `chunks` is a list of
    (col_start, width, class_idx) covering [0, N) in label-sorted column
    space, class-contiguous, each width <= CHUNK."""
    from contextlib import ExitStack

    import concourse.bass as bass
    import concourse.mybir as mybir
    import concourse.tile as tile

    try:
        from bass_rust import ScopedClock
    except ImportError:
        from concourse.vector_clock import ScopedClock

    f32 = mybir.dt.float32
    bf16 = mybir.dt.bfloat16
    AF = mybir.ActivationFunctionType
    ALU = mybir.AluOpType
    X = mybir.AxisListType.X
    TC = _split_drain_tile_context(tile, mybir, ScopedClock)

    nch = len(chunks)

    nc = bass.Bass("TRN2", target_bir_lowering=False, debug=False,
                   num_devices=NCORES)
    et_d = nc.dram_tensor("et", [D, N], bf16, kind="ExternalInput").ap()
    a2_d = nc.dram_tensor("anch2", [D, RPC], bf16, kind="ExternalInput").ap()
    au_d = nc.dram_tensor("anchu", [D, RPC], bf16, kind="ExternalInput").ap()
    pm_d = nc.dram_tensor("pmask", [BPB, NBLK, nch], f32, kind="ExternalInput").ap()
    nm_d = nc.dram_tensor("nmask", [BPB, NBLK, nch], f32, kind="ExternalInput").ap()
    dm_d = nc.dram_tensor("dmask", [BPB, NBLK, nch], f32, kind="ExternalInput").ap()
    id_d = nc.dram_tensor("ident", [D, BPB], f32, kind="ExternalInput").ap()
    out_d = nc.dram_tensor("out", [1, 1], f32, kind="ExternalOutput").ap()

    with TC(nc) as tc, ExitStack() as ctx:
        singles = ctx.enter_context(tc.tile_pool(name="singles", bufs=1))
        ps = ctx.enter_context(tc.tile_pool(name="ps", bufs=4, space="PSUM"))
        scr = ctx.enter_context(tc.tile_pool(name="scr", bufs=3))

        # anch2 feeds the very first matmul -- DMA it before the big et
        # transfer so the PE isn't stuck behind 2 MB of queue traffic.
        sb_a2 = singles.tile([D, RPC], bf16)
        nc.sync.dma_start(out=sb_a2, in_=a2_d)
        sb_et = singles.tile([D, N], bf16)
        for j in range(8):
            nc.sync.dma_start(out=sb_et[:, j * 1024:(j + 1) * 1024],
                              in_=et_d[:, j * 1024:(j + 1) * 1024])
        sb_au = singles.tile([D, RPC], bf16)
        nc.sync.dma_start(out=sb_au, in_=au_d)
        sb_pm = singles.tile([BPB, NBLK, nch], f32)
        nc.sync.dma_start(out=sb_pm, in_=pm_d)
        sb_nm = singles.tile([BPB, NBLK, nch], f32)
        nc.sync.dma_start(out=sb_nm, in_=nm_d)
        sb_dm = singles.tile([BPB, NBLK, nch], f32)
        nc.sync.dma_start(out=sb_dm, in_=dm_d)
        sb_id = singles.tile([D, BPB], f32)
        nc.sync.dma_start(out=sb_id, in_=id_d)

        acc = singles.tile([BPB, NBLK, nch], f32)     # per-chunk exp sums
        nmt = singles.tile([BPB, NBLK, nch], f32)     # per-chunk -(chunk max)
        nmv = singles.tile([BPB, NBLK], f32)          # -(block row max)
        cin = singles.tile([BPB, NBLK, nch], f32)     # nmt - nM  (corr input)
        corr = singles.tile([BPB, NBLK, nch], f32)    # exp(m_k - M)
        acct = singles.tile([BPB, NBLK, nch], f32)    # corrected chunk sums
        prod = singles.tile([BPB, NBLK, nch], f32)
        md = singles.tile([BPB, NBLK, BPB], f32)
        seltmp = singles.tile([BPB, NBLK, nch], f32)
        nm_sel = singles.tile([BPB, NBLK], f32)
        corr_sel = singles.tile([BPB, NBLK], f32)
        ppre = singles.tile([BPB, NBLK], f32)
        gsum = singles.tile([BPB, NBLK], f32)
        dvec = singles.tile([BPB, NBLK], f32)
        dsh = singles.tile([BPB, NBLK], f32)
        dexp = singles.tile([BPB, NBLK], f32)
        self_e = singles.tile([BPB, NBLK], f32)
        pnum = singles.tile([BPB, NBLK], f32)
        tden = singles.tile([BPB, NBLK], f32)
        numv = singles.tile([BPB, NBLK], f32)
        denv = singles.tile([BPB, NBLK], f32)
        ldv = singles.tile([BPB, NBLK], f32)
        lnv = singles.tile([BPB, NBLK], f32)
        lvec = singles.tile([BPB, NBLK], f32)
        lsum = singles.tile([BPB, 1], f32)
        ones = singles.tile([BPB, 1], f32)
        res_sb = singles.tile([1, 1], f32)

        for b in range(NBLK):
            lhs = sb_a2[:, b * BPB:(b + 1) * BPB]

            # one pass: per chunk, matmul -> chunk row-max -> exp+sum
            for k, (cs, w, _ci) in enumerate(chunks):
                pt = ps.tile([BPB, CHUNK], f32, tag="pst")
                off = 0
                while off < w:
                    sw = min(MM_MAX, w - off)
                    nc.tensor.matmul(pt[:, off:off + sw], lhs,
                                     sb_et[:, cs + off:cs + off + sw],
                                     start=True, stop=True)
                    off += sw
                nc.vector.reduce_max(nmt[:, b, k:k + 1], pt[:, :w],
                                     axis=X, negate=True)
                sc = scr.tile([BPB, CHUNK], bf16, tag="scrt")
                nc.scalar.activation(out=sc[:, :w], in_=pt[:, :w],
                                     func=AF.Exp,
                                     bias=nmt[:, b, k:k + 1], scale=1.0,
                                     accum_out=acc[:, b, k:k + 1])
            # diagonal self-term raw values (same products as the chunk pass)
            dps = ps.tile([BPB, CHUNK], f32, tag="pst")
            nc.tensor.matmul(dps[:, :BPB], lhs,
                             sb_au[:, b * BPB:(b + 1) * BPB],
                             start=True, stop=True)
            nc.vector.tensor_tensor(md[:, b, :], dps[:, :BPB], sb_id,
                                    op=ALU.mult)
            # -(row max) = min_k nmt
            nc.vector.tensor_reduce(nmv[:, b:b + 1], nmt[:, b, :],
                                    axis=X, op=ALU.min)
            # corr input: nmt - nM  (ACT applies scale=-1 -> exp(m_k - M) <= 1)
            nc.vector.tensor_scalar(cin[:, b, :], nmt[:, b, :],
                                    nmv[:, b:b + 1], None, op0=ALU.subtract)

        # ---- epilogue (batched over all blocks) ----
        nc.scalar.activation(out=corr, in_=cin, func=AF.Exp, scale=-1.0)
        nc.vector.tensor_tensor(acct, acc, corr, op=ALU.mult)

        # diag self-term, rescaled exactly like its in-sum contribution
        nc.vector.reduce_sum(dvec, md, axis=X)
        nc.vector.tensor_tensor(seltmp, nmt, sb_dm, op=ALU.mult)
        nc.vector.reduce_sum(nm_sel, seltmp, axis=X)
        nc.vector.tensor_tensor(seltmp, corr, sb_dm, op=ALU.mult)
        nc.vector.reduce_sum(corr_sel, seltmp, axis=X)
        nc.vector.tensor_tensor(dsh, dvec, nm_sel, op=ALU.add)
        nc.scalar.activation(out=dexp, in_=dsh, func=AF.Exp)
        nc.vector.tensor_tensor(self_e, dexp, corr_sel, op=ALU.mult)

        nc.vector.tensor_tensor(prod, acct, sb_pm, op=ALU.mult)
        nc.vector.reduce_sum(ppre, prod, axis=X)
        nc.vector.tensor_tensor(prod, acct, sb_nm, op=ALU.mult)
        nc.vector.reduce_sum(gsum, prod, axis=X)

        nc.vector.tensor_tensor(pnum, ppre, self_e, op=ALU.subtract)
        nc.vector.tensor_scalar_add(numv, pnum, EPS)
        nc.vector.tensor_tensor(tden, pnum, gsum, op=ALU.add)
        nc.vector.tensor_scalar_add(denv, tden, EPS)
        nc.scalar.activation(out=ldv, in_=denv, func=AF.Ln)
        nc.scalar.activation(out=lnv, in_=numv, func=AF.Ln)
        nc.vector.tensor_tensor(lvec, ldv, lnv, op=ALU.subtract)

        nc.vector.reduce_sum(lsum, lvec, axis=X)
        nc.vector.memset(ones, 1.0)
        fps = ps.tile([BPB, CHUNK], f32, tag="pst")
        nc.tensor.matmul(fps[:1, :1], lsum, ones, start=True, stop=True)
        nc.scalar.copy(res_sb, fps[:1, :1])
        nc.sync.dma_start(out=out_d, in_=res_sb)

    return nc


def _host_prepare(labels, embeddings):
    """Sort by label, build per-core input maps + the chunk schedule."""
    import ml_dtypes

    labels = np.asarray(labels).astype(np.int64)
    emb = np.asarray(embeddings, dtype=np.float32)
    assert labels.shape == (N,) and emb.shape == (N, D)

    order = np.argsort(labels, kind="stable")
    lab_s = labels[order]
    bounds = [0,
              int(np.searchsorted(lab_s, 0, side="left")),
              int(np.searchsorted(lab_s, 1, side="left")),
              N]

    chunks = []
    for ci in range(3):
        s, e = bounds[ci], bounds[ci + 1]
        npieces = max(1, -(-(e - s) // CHUNK))
        for p in range(npieces):
            a = s + (e - s) * p // npieces
            b = s + (e - s) * (p + 1) // npieces
            if b > a:
                chunks.append((a, b - a, ci))
    nch = len(chunks)

    et16 = np.ascontiguousarray(emb[order].T).astype(ml_dtypes.bfloat16)
    et2 = (et16.astype(np.float32) * 2.0).astype(ml_dtypes.bfloat16)  # exact

    in_maps = []
    for c in range(NCORES):
        lab_core = lab_s[c * RPC:(c + 1) * RPC].reshape(NBLK, BPB)
        L = lab_core.T                       # [p, b]
        ci_idx = (L + 1).astype(np.int64)    # class index 0/1/2
        pm3 = np.zeros((BPB, NBLK, 3), np.float32)
        pm3[:, :, 1] = 1.0                   # label-0 targets always positive
        np.put_along_axis(pm3, ci_idx[:, :, None], 1.0, axis=2)
        pm3[L == 0] = 1.0                    # label-0 anchors: all positive
        nm3 = np.zeros((BPB, NBLK, 3), np.float32)
        np.put_along_axis(nm3, (2 - ci_idx)[:, :, None], 1.0, axis=2)
        nm3[L == 0] = 0.0                    # label-0 anchors: no negatives
        chunk_cls = np.array([ci for (_, _, ci) in chunks])
        pm = np.ascontiguousarray(pm3[:, :, chunk_cls])   # [BPB, NBLK, nch]
        nm = np.ascontiguousarray(nm3[:, :, chunk_cls])

        # which chunk holds each anchor's diagonal column
        dmask = np.zeros((BPB, NBLK, nch), np.float32)
        for b in range(NBLK):
            g0 = c * RPC + b * BPB
            for k, (cs, w, _ci) in enumerate(chunks):
                gcols = np.arange(g0, g0 + BPB)
                hit = (gcols >= cs) & (gcols < cs + w)
                dmask[hit, b, k] = 1.0

        in_maps.append({
            "et": et16,
            "anch2": np.ascontiguousarray(et2[:, c * RPC:(c + 1) * RPC]),
            "anchu": np.ascontiguousarray(et16[:, c * RPC:(c + 1) * RPC]),
            "pmask": pm,
            "nmask": nm,
            "dmask": dmask,
            "ident": np.eye(BPB, dtype=np.float32),
        })

    # valid-anchor count, exactly as the reference's `valid` defines it
    n_cls = np.array([bounds[1], bounds[2] - bounds[1], N - bounds[2]])
    pos_counts = np.where(lab_s == 0, N - 1, n_cls[(lab_s + 1)] - 1 + n_cls[1])
    count = int((pos_counts > 0).sum())

    return chunks, in_maps, count


def _ensure_ntff_hook():
    """Register a stand-in ``antenv.axon_hooks`` if the image lacks it.

    ``run_bass_kernel_spmd(trace=True)`` under axon imports
    ``antenv.axon_hooks.get_axon_ntff_profile_hook`` unguarded; this image's
    ``antenv`` has no ``axon_hooks`` submodule, so tracing would crash.
    Provide the hook via direct ctypes calls into libaxon_pjrt.so (same C ABI
    the boot shim uses); if the .so or symbols are missing the getter returns
    None and concourse degrades to running without a trace."""
    import contextlib
    import ctypes
    import sys
    import types

    try:
        import antenv.axon_hooks  # noqa: F401
        return
    except ImportError:
        pass

    mod = types.ModuleType("antenv.axon_hooks")
    holder = [None]
    mod.set_axon_ntff_profile_hook = lambda h: holder.__setitem__(0, h)
    mod.get_axon_ntff_profile_hook = lambda: holder[0]

    try:
        lib = ctypes.CDLL("/opt/axon/libaxon_pjrt.so")
        if hasattr(lib, "axon_start_nrt_profile"):
            lib.axon_start_nrt_profile.argtypes = [
                ctypes.POINTER(ctypes.c_int64), ctypes.c_size_t]
            lib.axon_start_nrt_profile.restype = ctypes.c_int64
            lib.axon_stop_nrt_profile.argtypes = [ctypes.c_char_p]
            lib.axon_stop_nrt_profile.restype = ctypes.c_int64

            @contextlib.contextmanager
            def _hook(output_dir, device_ids):
                import jax
                jax.devices()
                if device_ids:
                    ids = (ctypes.c_int64 * len(device_ids))(*device_ids)
                    rc = lib.axon_start_nrt_profile(ids, len(device_ids))
                else:
                    rc = lib.axon_start_nrt_profile(None, 0)
                if rc != 0:
                    raise RuntimeError(f"axon_start_nrt_profile rc={rc}")
                try:
                    yield
                finally:
                    n = lib.axon_stop_nrt_profile(str(output_dir).encode())
                    if n < 0:
                        raise RuntimeError(f"axon_stop_nrt_profile rc={n}")

            holder[0] = _hook
    except OSError:
        pass

    sys.modules["antenv.axon_hooks"] = mod
    try:
        import antenv
        antenv.axon_hooks = mod
    except ImportError:
        pass


def kernel(labels, embeddings, **_unused):
    global LAST_RESULT
    _ensure_ntff_hook()
    from concourse.bass_utils import run_bass_kernel_spmd

    chunks, in_maps, count = _host_prepare(labels, embeddings)
    # The kernel sums the per-anchor losses of *every* anchor; that matches
    # the reference's where(valid, loss, 0) only when every anchor is valid
    # (guaranteed for this problem's label distribution -- each class and
    # class 0 are nonempty).
    assert count == N, "kernel assumes all anchors valid"

    nc = _build_program(chunks)
    res = run_bass_kernel_spmd(nc, in_maps, core_ids=list(range(NCORES)))
    LAST_RESULT = res

    total = np.float32(0.0)
    for i in range(NCORES):
        total = np.float32(total + np.float32(res.results[i]["out"].reshape(())))
    mean = np.float32(total / np.float32(max(count, 1)))
    outv = mean if count > 0 else np.float32(0.0)
    return np.array(outv, dtype=np.float32)



# revision 3
# speedup vs baseline: 1.3499x; 1.3499x over previous
"""Trainium2 Bass kernel for nn_ContrastiveEmbeddingLoss (N=8192, D=128).

Scheme ("column-attributed symmetric sums", v2):

Rows (anchors) are sharded 1024/core.  Labels are sorted on host so the
three classes {-1, 0, +1} occupy contiguous column ranges.  Key algebraic
facts exploited:

1. label-0 anchors have an empty negative set, so their loss is exactly 0.
   Their per-row sums are never needed => the entire class-0 COLUMN stripe
   is skipped (columns only; class-0 rows still feed other anchors' sums).

2. With the stabilizer bias b_i = -(o_i + O)/2 (o_i = 2||e_i||^2 >= row max
   by Cauchy-Schwarz, O = max_i o_i) and mask weights u_i = exp((o_i-O)/2):

       sum_i u_i * exp(sim_ij + b_i)  =  sum_i exp(sim_ij - O)

   which by symmetry of sim is the CLASS-RESOLVED ROW SUM of anchor j
   scaled by exp(o_j - O).  So per-class column sums computed by tiny
   [128x4] mask-matmuls on the TensorE (contraction over the partition
   axis) replace all row-direction accumulation: no reduce_max, no
   activation accumulators, no per-chunk rescaling.

Per core, per 128-row block: sim matmuls (lhsT = 2*bf16(E) block) fill
rotating [128,1024] PSUM slots; one wide ScalarE exp per slot (bf16 out);
one mask-matmul per 512-column window accumulates class sums into a
persistent PSUM region (pre-zeroed by zero-weight matmuls so start=False
accumulation is well-defined).  A separate diagonal mini-matmul recomputes
sim_rr bitwise-identically; its exp'd bf16 value is extracted with an
identity mask on the VectorE so the host can subtract the self term with
exact cancellation (the value subtracted is bit-identical to the addend
inside the PE accumulation, and f32 sums of non-negatives are monotone,
so P >= 0 always).

Device outputs per core: colp [16, COLW] f32 (class-partial column sums,
packed 4 partition-groups x 4-class rows) and dvecy [128, 8] f32 (exp'd
diagonal).  Host (f64): all-core reduce of colp (~100 KB), per-anchor
  S_c[j] = e^{O-o_j} * colp[c, j],   P = S_{c_j} + S_0 - self,
  G = S_{-c_j},  loss = log(P+G+eps) - log(P+eps),  mean over N.
"""

import numpy as np

N, D = 8192, 128
NCORES = 8
RPC = N // NCORES        # anchor rows per core
BPB = 128                # rows per block (= partition dim)
NBLK = RPC // BPB        # blocks per core
TEMPERATURE = 0.5
EPS = 1e-08
WIN = 512                # column window (= one PSUM bank of f32)
FILLW = 1024             # PSUM fill slot width (2 windows)

LAST_RESULT = None       # BassKernelResults of the most recent run (for test.py)


def _split_drain_tile_context(tile_mod, mybir, ScopedClock):
    """TileContext subclass that never emits more than one sync wait per
    instruction -- the walrus build here rejects any instruction carrying
    more than one ("Too many sync wait commands").  Excess waits are hoisted
    onto same-engine NoOp instructions inserted immediately before (engine
    program order makes sequential single waits equivalent to one multi-wait:
    a logical AND), and the tail drain is split into sequential drains."""

    class SplitWaitTileContext(tile_mod.TileContext):
        def _lower_ordered_insts(self, ordered):
            unassigned = mybir.EngineType.Unassigned
            for insts in ordered.values():
                new_list = []
                changed = False
                for inst in insts:
                    si = inst.sync_info
                    waits = list(si.on_wait) if si is not None and si.on_wait else []
                    eng = getattr(inst, "engine", None)
                    if len(waits) > 1 and eng is not None and eng != unassigned:
                        keep = [w for w in waits if w.sync_type != "semaphore"]
                        move = [w for w in waits if w.sync_type == "semaphore"]
                        if not keep and move:
                            keep = [move.pop()]
                        for w in move:
                            nop = mybir.InstNoOp(
                                name=f"I-{self.nc.next_id()}", ins=[], outs=[]
                            )
                            nop.engine = eng
                            nop.sync_info = mybir.SyncInfo(
                                on_wait=[w], on_update=[]
                            )
                            new_list.append(nop)
                        inst.sync_info = mybir.SyncInfo(
                            on_wait=keep,
                            on_update=list(si.on_update) if si.on_update else [],
                        )
                        changed = True
                    new_list.append(inst)
                if changed:
                    insts[:] = new_list
            return super()._lower_ordered_insts(ordered)

        def _drain_and_barrier(self, tick_clock, wait_clock):
            nc = self.nc
            drain_inst = nc.sync.drain()
            wait_clock.add_sem_waits(
                drain_inst.ins, ScopedClock({None: tick_clock.global_clock})
            )
            si = drain_inst.ins.sync_info
            waits = list(si.on_wait) if si is not None and si.on_wait else []
            if len(waits) > 1:
                drain_inst.ins.sync_info = mybir.SyncInfo(
                    on_wait=waits[:1],
                    on_update=list(si.on_update) if si.on_update else [],
                )
                for i in range(1, len(waits)):
                    extra = nc.sync.drain()
                    extra.ins.sync_info = mybir.SyncInfo(
                        on_wait=waits[i : i + 1], on_update=[]
                    )
            # Single-shot NEFF: skip the semaphore-clearing pass + second
            # barrier (cleanup for NEFF re-execution, which never happens
            # here -- each kernel() call compiles and runs a fresh NEFF).
            nc.all_engine_barrier()
            assert self.sems is not None
            popped = nc._tile_sem_poison_stack.pop()
            assert popped is self._sem_poison
            # Sems are intentionally NOT cleared or returned to the pool:
            # this is the outermost (only) TileContext of a one-shot program,
            # so nothing after it allocates semaphores.

    return SplitWaitTileContext


class _Sched:
    """Global (core-independent) column schedule.

    kept:  list of windows (gw, a, w, off, g, f):
      gw = global 512-window index, [a, a+w) = the kept (non-class-0)
      columns inside it, off = start of this window's columns in the packed
      etnz tensor, (g, f) = colp partition-group and free offset.
    fills: list of lists of kept-indices (<= 2 per fill, slot offsets
      512*pos within the fill).
    """

    def __init__(self, b1, b2):
        self.b1, self.b2 = b1, b2
        keep_ranges = [(0, b1), (b2, N)]
        kept = []
        off = 0
        for gw in range(N // WIN):
            lo, hi = gw * WIN, (gw + 1) * WIN
            for (ra, rb) in keep_ranges:
                a, b = max(lo, ra), min(hi, rb)
                if b > a:
                    kept.append([gw, a, b - a, off])
                    off += b - a
        # a 512-window can intersect both keep ranges only if the class-0
        # stripe is narrower than 512 columns; assert it can't happen
        gws = [k[0] for k in kept]
        assert len(set(gws)) == len(gws), "window split by narrow class-0"
        self.KC = off
        for i, k in enumerate(kept):
            k.append(i % 4)            # g: partition group
            k.append((i // 4) * WIN)   # f: colp free offset
        self.kept = kept
        self.KW = len(kept)
        self.COLW = WIN * ((self.KW + 3) // 4)
        self.fills = [list(range(i, min(i + 2, self.KW)))
                      for i in range(0, self.KW, 2)]


def _build_program(sched):
    from contextlib import ExitStack

    import concourse.bass as bass
    import concourse.mybir as mybir
    import concourse.tile as tile

    try:
        from bass_rust import ScopedClock
    except ImportError:
        from concourse.vector_clock import ScopedClock

    f32 = mybir.dt.float32
    bf16 = mybir.dt.bfloat16
    AF = mybir.ActivationFunctionType
    ALU = mybir.AluOpType
    X = mybir.AxisListType.X
    TC = _split_drain_tile_context(tile, mybir, ScopedClock)

    KC, KW, COLW = sched.KC, sched.KW, sched.COLW

    nc = bass.Bass("TRN2", target_bir_lowering=False, debug=False,
                   num_devices=NCORES)
    etnz_d = nc.dram_tensor("etnz", [D, KC], bf16, kind="ExternalInput").ap()
    e2o_d = nc.dram_tensor("et2own", [D, RPC], bf16, kind="ExternalInput").ap()
    edg_d = nc.dram_tensor("etdiag", [D, RPC], bf16, kind="ExternalInput").ap()
    wm_d = nc.dram_tensor("wmask", [BPB, NBLK * 4], bf16, kind="ExternalInput").ap()
    bs_d = nc.dram_tensor("biasb", [BPB, NBLK], f32, kind="ExternalInput").ap()
    id_d = nc.dram_tensor("ident", [BPB, BPB], bf16, kind="ExternalInput").ap()
    zr_d = nc.dram_tensor("zeros", [D, WIN], bf16, kind="ExternalInput").ap()
    colp_d = nc.dram_tensor("colp", [16, COLW], f32, kind="ExternalOutput").ap()
    dvy_d = nc.dram_tensor("dvecy", [BPB, NBLK], f32, kind="ExternalOutput").ap()

    with TC(nc) as tc, ExitStack() as ctx:
        singles = ctx.enter_context(tc.tile_pool(name="singles", bufs=1))
        ps = ctx.enter_context(tc.tile_pool(name="ps", bufs=1, space="PSUM"))
        scr = ctx.enter_context(tc.tile_pool(name="scr", bufs=1))

        # small tensors first (cheap, unblock early compute), etnz behind
        sb_bs = singles.tile([BPB, NBLK], f32)
        nc.sync.dma_start(out=sb_bs, in_=bs_d)
        sb_e2o = singles.tile([D, RPC], bf16)
        nc.sync.dma_start(out=sb_e2o, in_=e2o_d)
        sb_edg = singles.tile([D, RPC], bf16)
        nc.sync.dma_start(out=sb_edg, in_=edg_d)
        sb_wm = singles.tile([BPB, NBLK * 4], bf16)
        nc.sync.dma_start(out=sb_wm, in_=wm_d)
        sb_id = singles.tile([BPB, BPB], bf16)
        nc.sync.dma_start(out=sb_id, in_=id_d)
        sb_zr = singles.tile([D, WIN], bf16)
        nc.sync.dma_start(out=sb_zr, in_=zr_d)
        sb_et = singles.tile([D, KC], bf16)
        for a in range(0, KC, 1024):
            w = min(1024, KC - a)
            nc.sync.dma_start(out=sb_et[:, a:a + w], in_=etnz_d[:, a:a + w])

        dvy_sb = singles.tile([BPB, NBLK], f32)

        # persistent per-class column-sum accumulator, pre-zeroed via
        # zero-weight matmuls (start=True clears has_written; value 0)
        colpart = ps.tile([BPB, COLW], f32, tag="colpart")
        for z in range(COLW // WIN):
            nc.tensor.matmul(colpart[:, z * WIN:(z + 1) * WIN],
                             sb_zr[:, :BPB], sb_zr,
                             start=True, stop=False, skip_group_check=True)

        for b in range(NBLK):
            lhs = sb_e2o[:, b * BPB:(b + 1) * BPB]
            wmb = sb_wm[:, 4 * b:4 * b + 4]
            bias = sb_bs[:, b:b + 1]

            for fi, fill in enumerate(sched.fills):
                pf = ps.tile([BPB, FILLW], f32, tag="fill", bufs=2)
                yf = scr.tile([BPB, FILLW], bf16, tag="yf", bufs=3)
                # sim matmuls: one per kept window in this fill
                for pos, ki in enumerate(fill):
                    gw, a, w, off, g, f = sched.kept[ki]
                    nc.tensor.matmul(pf[:, pos * WIN:pos * WIN + w],
                                     lhs, sb_et[:, off:off + w],
                                     start=True, stop=True)
                # exp over the valid runs of this fill (merge when the
                # first window is full so its data abuts the second slot)
                runs = []
                for pos, ki in enumerate(fill):
                    w = sched.kept[ki][2]
                    if runs and runs[-1][0] + runs[-1][1] == pos * WIN:
                        runs[-1][1] += w
                    else:
                        runs.append([pos * WIN, w])
                for (ra, rw) in runs:
                    nc.scalar.activation(out=yf[:, ra:ra + rw],
                                         in_=pf[:, ra:ra + rw],
                                         func=AF.Exp, bias=bias, scale=1.0)
                # per-class column sums (contract over the 128 rows)
                for pos, ki in enumerate(fill):
                    gw, a, w, off, g, f = sched.kept[ki]
                    nc.tensor.matmul(
                        colpart[32 * g:32 * g + 4, f:f + w],
                        wmb, yf[:, pos * WIN:pos * WIN + w],
                        start=False, stop=(b == NBLK - 1 and ki == KW - 1),
                        skip_group_check=True, tile_position=(0, 32 * g))

            # diagonal: recompute sim_rr bitwise-identically, exp to bf16
            # (same rounding as the in-sum Y), extract with identity mask
            pd = ps.tile([BPB, BPB], f32, tag="dfill", bufs=1)
            nc.tensor.matmul(pd, lhs, sb_edg[:, b * BPB:(b + 1) * BPB],
                             start=True, stop=True)
            dy = scr.tile([BPB, BPB], bf16, tag="dy", bufs=2)
            nc.scalar.activation(out=dy, in_=pd, func=AF.Exp,
                                 bias=bias, scale=1.0)
            md = scr.tile([BPB, BPB], f32, tag="md", bufs=2)
            nc.vector.tensor_tensor(md, dy, sb_id, op=ALU.mult)
            nc.vector.reduce_sum(dvy_sb[:, b:b + 1], md, axis=X)

        # evacuate colpart and ship results
        colsb = singles.tile([BPB, COLW], f32)
        nc.vector.tensor_copy(colsb, colpart)
        for g in range(4):
            nc.sync.dma_start(out=colp_d[4 * g:4 * g + 4, :],
                              in_=colsb[32 * g:32 * g + 4, :])
        nc.sync.dma_start(out=dvy_d, in_=dvy_sb)

    return nc


def _host_prepare(labels, embeddings):
    """Sort by label, build the global schedule + per-core input maps."""
    import ml_dtypes

    labels = np.asarray(labels).astype(np.int64)
    emb = np.asarray(embeddings, dtype=np.float32)
    assert labels.shape == (N,) and emb.shape == (N, D)

    order = np.argsort(labels, kind="stable")
    lab_s = labels[order]
    b1 = int(np.searchsorted(lab_s, 0, side="left"))
    b2 = int(np.searchsorted(lab_s, 1, side="left"))
    assert 0 < b1 < b2 < N, "kernel assumes all three classes nonempty"
    assert b2 - b1 >= WIN, "kernel assumes class-0 stripe >= one window"

    sched = _Sched(b1, b2)

    eb16 = emb[order].astype(ml_dtypes.bfloat16)          # [N, D] bf16
    ebf = eb16.astype(np.float32)
    et = np.ascontiguousarray(ebf.T).astype(ml_dtypes.bfloat16)  # [D, N]
    et2 = (et.astype(np.float32) * 2.0).astype(ml_dtypes.bfloat16)  # exact

    o = 2.0 * (ebf.astype(np.float64) ** 2).sum(axis=1)   # [N] f64
    O = float(o.max())
    bias = (-(o + O) / 2.0).astype(np.float32)
    u16 = np.exp((o - O) / 2.0).astype(np.float32).astype(ml_dtypes.bfloat16)
    cls = (lab_s + 1).astype(np.int64)                    # 0,1,2

    etnz = np.empty((D, sched.KC), dtype=ml_dtypes.bfloat16)
    for (gw, a, w, off, g, f) in sched.kept:
        etnz[:, off:off + w] = et[:, a:a + w]
    etnz = np.ascontiguousarray(etnz)

    in_maps = []
    for c in range(NCORES):
        rows = slice(c * RPC, (c + 1) * RPC)
        wm = np.zeros((BPB, NBLK * 4), np.float32)
        for b in range(NBLK):
            rr = np.arange(c * RPC + b * BPB, c * RPC + (b + 1) * BPB)
            wm[np.arange(BPB), 4 * b + cls[rr]] = u16[rr].astype(np.float32)
        in_maps.append({
            "etnz": etnz,
            "et2own": np.ascontiguousarray(et2[:, rows]),
            "etdiag": np.ascontiguousarray(et[:, rows]),
            "wmask": wm.astype(ml_dtypes.bfloat16),
            "biasb": np.ascontiguousarray(
                bias[rows].reshape(NBLK, BPB).T),
            "ident": np.eye(BPB, dtype=np.float32).astype(ml_dtypes.bfloat16),
            "zeros": np.zeros((D, WIN), np.float32).astype(ml_dtypes.bfloat16),
        })

    host = {
        "order": order, "lab_s": lab_s, "cls": cls, "b1": b1, "b2": b2,
        "o": o, "O": O, "u32": u16.astype(np.float32), "sched": sched,
    }
    return sched, in_maps, host


def _host_epilogue(host, colps, dvecys):
    """Combine per-core partials into the scalar mean loss (f64)."""
    sched = host["sched"]
    cls, o, O, u32 = host["cls"], host["o"], host["O"], host["u32"]

    # f32 monotone reduction across cores preserves sum >= self-term
    colp = np.zeros_like(colps[0], dtype=np.float32)
    for cp in colps:
        colp = colp + cp.astype(np.float32)

    # per-class sums S3[c, j] (still scaled by exp(-O)), j in sorted space
    S3 = np.zeros((3, N), np.float32)
    valid = np.zeros(N, bool)
    for (gw, a, w, off, g, f) in sched.kept:
        for c in range(3):
            S3[c, a:a + w] = colp[4 * g + c, f:f + w]
        valid[a:a + w] = True

    # exp'd diagonal per sorted anchor (bf16 value as f32)
    dvy = np.concatenate(
        [np.asarray(d, np.float32).T.reshape(-1) for d in dvecys])  # [N]

    j = np.nonzero(valid)[0]                      # all +-1 anchors
    cj = cls[j]
    selfp = (u32[j] * dvy[j]).astype(np.float32)  # exact f32 product
    own = S3[cj, j]                               # includes the self term
    ppre = np.maximum(own.astype(np.float32) - selfp, np.float32(0.0))
    pos = ppre.astype(np.float64) + S3[1, j].astype(np.float64)
    neg = S3[2 - cj, j].astype(np.float64)
    scale = np.exp(O - o[j])                      # f64, may be huge
    P = scale * pos
    G = scale * neg
    loss = np.log(P + G + EPS) - np.log(P + EPS)
    return np.float32(loss.sum() / N)


def _ensure_ntff_hook():
    """Register a stand-in ``antenv.axon_hooks`` if the image lacks it.

    ``run_bass_kernel_spmd(trace=True)`` under axon imports
    ``antenv.axon_hooks.get_axon_ntff_profile_hook`` unguarded; this image's
    ``antenv`` has no ``axon_hooks`` submodule, so tracing would crash.
    Provide the hook via direct ctypes calls into libaxon_pjrt.so (same C ABI
    the boot shim uses); if the .so or symbols are missing the getter returns
    None and concourse degrades to running without a trace."""
    import contextlib
    import ctypes
    import sys
    import types

    try:
        import antenv.axon_hooks  # noqa: F401
        return
    except ImportError:
        pass

    mod = types.ModuleType("antenv.axon_hooks")
    holder = [None]
    mod.set_axon_ntff_profile_hook = lambda h: holder.__setitem__(0, h)
    mod.get_axon_ntff_profile_hook = lambda: holder[0]

    try:
        lib = ctypes.CDLL("/opt/axon/libaxon_pjrt.so")
        if hasattr(lib, "axon_start_nrt_profile"):
            lib.axon_start_nrt_profile.argtypes = [
                ctypes.POINTER(ctypes.c_int64), ctypes.c_size_t]
            lib.axon_start_nrt_profile.restype = ctypes.c_int64
            lib.axon_stop_nrt_profile.argtypes = [ctypes.c_char_p]
            lib.axon_stop_nrt_profile.restype = ctypes.c_int64

            @contextlib.contextmanager
            def _hook(output_dir, device_ids):
                import jax
                jax.devices()
                if device_ids:
                    ids = (ctypes.c_int64 * len(device_ids))(*device_ids)
                    rc = lib.axon_start_nrt_profile(ids, len(device_ids))
                else:
                    rc = lib.axon_start_nrt_profile(None, 0)
                if rc != 0:
                    raise RuntimeError(f"axon_start_nrt_profile rc={rc}")
                try:
                    yield
                finally:
                    n = lib.axon_stop_nrt_profile(str(output_dir).encode())
                    if n < 0:
                        raise RuntimeError(f"axon_stop_nrt_profile rc={n}")

            holder[0] = _hook
    except OSError:
        pass

    sys.modules["antenv.axon_hooks"] = mod
    try:
        import antenv
        antenv.axon_hooks = mod
    except ImportError:
        pass


def kernel(labels, embeddings, **_unused):
    global LAST_RESULT
    _ensure_ntff_hook()
    from concourse.bass_utils import run_bass_kernel_spmd

    sched, in_maps, host = _host_prepare(labels, embeddings)
    nc = _build_program(sched)
    res = run_bass_kernel_spmd(nc, in_maps, core_ids=list(range(NCORES)))
    LAST_RESULT = res

    colps = [res.results[i]["colp"] for i in range(NCORES)]
    dvecys = [res.results[i]["dvecy"] for i in range(NCORES)]
    return np.array(_host_epilogue(host, colps, dvecys), dtype=np.float32)
